# revision 45
# baseline (speedup 1.0000x reference)
"""Trainium2 Bass kernel for nn_F0ProcessorCell — fused custom-DVE scan.

Reference semantics (per lane b, scanned over t):
    a_t = clamp(x_t, 0, 1)                      # note_activity
    r_t = clamp(s_{t-1} - thr, 0, 1)            # release_end, thr = rd*250
    n_t = a_t*x_t + (1-a_t)*n_{t-1}*(1-r_t)
    s_t = (s_{t-1}+1)*(1-a_t)*(1-r_t)
    out[b,t] = n_t

Fast path (guarded on CPU, exact): when every run of consecutive x<1 is
<= thr steps, r_t == 0 identically and the cell is the first-order
linear scan
    n_t = u_t*n_{t-1} + c_t,  a=clamp01(x), u=1-a, c=a*x.

This kernel computes THE WHOLE CELL in ONE hand-authored custom DVE
(VectorE) instruction per tile.  The stock pipeline needed ScalarE
relu/act + DVE STT + DVE tensor_tensor_scan (3.2 cyc/col); the fused op
reads raw f32 x and runs at the scan's intrinsic feedback rate
(2 cyc/col), writing fp16 directly:

uop program (v3, 8 ALU blocks; config travels WITH each element):
  uops[0] SEED   (1 cycle, no consume):  s4: flop4 <- CONST_0 (chunk carry)
  uops[1] ELEM A (consumes one x column):
      s0: r = MAX(x, 0)         s1: a = MIN(r, 1)
      s2: c = MULT(a, x)  [capture a -> lane4]
      s3: u = SUB(1, a)         s4: m = MULT(u, CURR=n_prev)
  uops[2] BUBBLE B (1 cycle, no consume, writes out):
      s2: BYPASS(CURR)=hold c   s3: BYPASS(PREV)=carry c
      s4: n = ADD(CURR=m, PREV=c)   s5..7: BYPASS -> WR0_LO (fp16)
A and B alternate: element e's MULT (cycle 2e+4) reads flop4=n_{e-1}
written by the previous bubble; bubble e's ADD (cycle 2e+5) combines
element e's m with the held c_e and rewrites flop4=n_e.  2 cycles per
element, exactly the stock scan's feedback rate, with all elementwise
prep riding the same pipeline for free.

y is stored fp16 (halves output HBM traffic; rel err ~1e-4 << the 2e-2
gate), upcast to f32 on host.  Falls back to an exact numpy scan if the
run-length guard fails.

Sharding: batch axis 0 (2048 lanes) split across 8 cores, 256 lanes per
core as 2 partition-groups of 128; time axis chunked, scan carry chained
across chunks via the previous out-tile's last column (fp16 [P,1] AP fed
to the seed uop through the CONST_0 slot).
"""

from dataclasses import dataclass

import numpy as np

from concourse import bacc, tile
from concourse import mybir
from concourse import dve_ops as _dve_ops_mod
from concourse.bass_utils import run_bass_kernel_spmd
from concourse.dve_ops import OPS, DveOp
from concourse.dve_spec import Spec, Src0, relu as _spec_relu
from concourse.dve_uop import (
    ENABLE,
    AluInp,
    AluOp,
    DelayInp,
    DveOpSpec,
    InpSel,
    OutPath,
    OutSel,
    Trigger,
    UopConfig,
)

N_CORES = 8
B, T = 2048, 16000
LPC = B // N_CORES          # 256 lanes per core
P = 128                     # SBUF partitions
GROUPS = LPC // P           # 2 partition-groups per core
F = 4000                    # max time-chunk (free-dim) size

_F32 = mybir.dt.float32
_F16 = mybir.dt.float16

_F0_NAME = "F0_FUSED_SCAN_ANT"

# delay lanes: 0=x(SRC_0) 1=ONE 2=ZERO 3=CONST_0(init) 4=a-capture
_LANES = (0, 1, 2, 3, 4)
_PREV = AluInp.PREV_ALU_OUT
_CURR = AluInp.CURR_ALU_OUT


def _lane(d):
    return AluInp(int(AluInp.PREV_DELAY_0) + d)


def _mk_uop(inps, stages, capture=None, write=False, trigger=(), nxt=(0, 0, 0),
            repeat=0, consume=False, out_a_from=None):
    u = UopConfig()
    for d, sel in inps:
        u.enable_input(sel, d + 1)          # input lane d+1 feeds delay chain d
    for st in range(8):
        u.datapath_config[st].pass_through_delay(*_LANES)
    for st in range(8):
        op, a, b = stages.get(st, (AluOp.BYPASS, _PREV, _PREV))
        u.datapath_config[st].enable_alu(op, a, b)
    if capture is not None:
        st, ln = capture
        u.datapath_config[st].enable_delay_from_src(DelayInp.PREV_ALU_OUT, ln)
    if write:
        u.enable_output(OutSel.ALU_OUT, OutPath.WR0_LO)
    if out_a_from is not None:
        for st in range(out_a_from, 8):
            u.datapath_config[st].alu_out_a_enable = ENABLE
    u.accum_enabled = ENABLE   # accum_out carries the final scan state (f32)
    u.trigger = tuple(trigger)
    u.next_uop = tuple(nxt)
    u.repeat_count = repeat
    u.require_inp0 = int(consume)
    return u


def _build_f0_uops():
    # All three uops share one input-lane config (framework convention:
    # seed and steady have identical inp sets); only require_inp0 differs.
    inps = [(0, InpSel.SRC_0), (1, InpSel.ONE_F32), (2, InpSel.ZERO),
            (3, InpSel.CONST_0)]
    seed = _mk_uop(
        inps=inps,
        stages={4: (AluOp.BYPASS, _lane(3), _lane(3))},  # flop4 <- init
        trigger=(Trigger.COUNT, Trigger.NONE, Trigger.NONE),
        nxt=(1, 0, 0), repeat=1,
    )
    # Consuming slot: elementwise prep + u*state; ALSO emits the PREVIOUS
    # element's n (held in flop5 by the interleaved bubble slots) -> the
    # output stream is shifted one element; the caller pads in0 with one
    # dummy tail column and drops out column 0.
    elem = _mk_uop(
        inps=inps,
        stages={
            0: (AluOp.MAX, _lane(0), _lane(2)),        # r = max(x, 0)
            1: (AluOp.MIN, _PREV, _lane(1)),           # a = min(r, 1)
            2: (AluOp.MULTIPLY, _PREV, _lane(0)),      # c = a * x
            3: (AluOp.SUBTRACT, _lane(1), _lane(4)),   # u = 1 - a
            4: (AluOp.MULTIPLY, _PREV, _CURR),         # m = u * n_prev
            5: (AluOp.BYPASS, _CURR, _CURR),           # pick up held n_prev
        },
        capture=(2, 4),                                # lane4 <- a (flop1)
        write=True,
        trigger=(Trigger.SRC_TENSOR_DONE, Trigger.COUNT, Trigger.NONE),
        nxt=(0, 2, 0), repeat=1, consume=True,
    )
    # Non-consuming bubble (seed-shaped): completes n = m + c one cycle
    # behind the element and parks it in flop5 for the next element slot.
    # out_a threads n down the a-flops so the LAST bubble leaves the final
    # state in block 7's a-flop -> accum_out ([P,1] f32 chunk carry).
    bubble = _mk_uop(
        inps=inps,
        stages={
            2: (AluOp.BYPASS, _CURR, _CURR),           # hold c
            4: (AluOp.ADD, _CURR, _PREV),              # n = m + c
        },
        out_a_from=4,
        trigger=(Trigger.COUNT, Trigger.NONE, Trigger.NONE),
        nxt=(1, 0, 0), repeat=1,
    )
    return [seed, elem, bubble]


def _f0_reference(in0, in1, s0, s1, imm2):
    x = np.asarray(in0, np.float32)
    a = np.clip(x, 0.0, 1.0)
    u = 1.0 - a
    c = a * x
    state = np.broadcast_to(np.asarray(s0, np.float32).reshape(-1, 1),
                            (x.shape[0], 1)).copy().reshape(-1)
    out = np.empty_like(x)
    for t in range(x.shape[-1]):
        state = u[..., t] * state + c[..., t]
        out[..., t] = state
    return out, state.reshape(-1, 1)


@dataclass(frozen=True)
class _HandDveOp(DveOp):
    """DveOp whose uop program is hand-authored (bypasses lower()/sha)."""

    def compile(self, ver):
        from concourse.dve_ops import _COMPILE_CACHE, get_dve_sub_opcode
        key = (self.name, ver)
        if (r := _COMPILE_CACHE.get(key)) is not None:
            return r
        assert ver == "v3", f"{self.name} is hand-built for TRN2 (v3) only"
        result = DveOpSpec(
            name=self.name,
            opcode=get_dve_sub_opcode(self.name),
            uops=_build_f0_uops(),
            rd1_en=False,
        )
        for u in result.uops:
            u.validate(ver)
        _COMPILE_CACHE[key] = result
        return result


def _register_f0_op():
    for o in OPS:
        if o.name == _F0_NAME:
            return o
    op = _HandDveOp(
        _F0_NAME,
        # body is a registration placeholder (never lowered — compile() is
        # overridden); `reference` carries the true semantics for CoreSim.
        # accum=ADD marks the op accum_out-bearing (the final scan state).
        Spec(body=_spec_relu(Src0), accum=AluOp.ADD,
             reference=_f0_reference),
        False,
        {},
    )
    OPS.append(op)
    _dve_ops_mod._SUB_OPCODE_FOR_NAME[_F0_NAME] = (
        _dve_ops_mod._CUSTOM_DVE_ROW_BASE + len(OPS) - 1
    )
    assert _dve_ops_mod._SUB_OPCODE_FOR_NAME[_F0_NAME] < 0x20
    _dve_ops_mod.CUSTOM_DVE_SPECS[_F0_NAME] = op.spec
    return op


_F0_OP = _register_f0_op()


def _build_nc():
    nc = bacc.Bacc("TRN2", target_bir_lowering=False, debug=False,
                   num_devices=N_CORES)
    x_ap = nc.dram_tensor("x", [LPC, T], _F32, kind="ExternalInput").ap()
    y_ap = nc.dram_tensor("y", [LPC, T], _F16, kind="ExternalOutput").ap()

    with tile.TileContext(nc) as tc:
        with (
            tc.tile_pool(name="xin", bufs=7) as pool_x,
            tc.tile_pool(name="nout", bufs=5) as pool_n,
            tc.tile_pool(name="carry", bufs=3) as pool_cr,
        ):
            from collections import deque
            prev = [None] * GROUPS
            pend = [deque() for _ in range(GROUPS)]

            # tapered prologue fills the pipeline early; split tail drains fast
            widths = [500, 500, 1000, 2000, 4000, 4000, 2000, 1000, 500, 500]
            assert sum(widths) == T
            segs, off = [], 0
            for w in widths:
                segs.append((off, w))
                off += w

            def emit_front(seg, g):
                off, w = seg
                rows = slice(g * P, (g + 1) * P)
                xt = pool_x.tile([P, F + 1], _F32, tag="x")
                nc.sync.dma_start(xt[:, 0:w], x_ap[rows, off:off + w])
                pend[g].append((xt, seg))

            def emit_back(g):
                xt, (off, w) = pend[g].popleft()
                rows = slice(g * P, (g + 1) * P)
                nt = pool_n.tile([P, F + 1], _F16, tag="n")
                init = 0.0 if prev[g] is None else prev[g]
                # output is shifted one column: in0 gets a dummy tail column
                # (stale SBUF data — any value is safe); out[:, 0] is the
                # re-emitted carry, real n_t land at out[:, 1:w+1].  The f32
                # chunk carry comes out of accum_out (a-flop threading).
                ct = pool_cr.tile([P, 1], _F32, tag="carry")
                nc.vector._custom_dve(_F0_OP, out=nt[:, 0:w + 1],
                                      in0=xt[:, 0:w + 1], s0=init,
                                      accum_out=ct[:, :])
                prev[g] = ct[:, :]
                nc.sync.dma_start(y_ap[rows, off:off + w], nt[:, 1:w + 1])

            LAG = 3
            NSEG = len(segs)
            for k in range(NSEG + LAG):
                for g in range(GROUPS):
                    if k >= LAG:
                        emit_back(g)
                    if k < NSEG:
                        emit_front(segs[k], g)
    nc.compile()
    return nc


_NC_CACHE = None


def _get_nc():
    global _NC_CACHE
    if _NC_CACHE is None:
        _NC_CACHE = _build_nc()
    return _NC_CACHE


def _max_run_length_lt1(x):
    """Max length, over all lanes, of a run of consecutive values < 1.0."""
    m = x < np.float32(1.0)                      # [B, T] bool
    cs = np.cumsum(m, axis=1, dtype=np.int64)
    reset = np.where(~m, cs, 0)
    run = cs - np.maximum.accumulate(reset, axis=1)
    run = np.where(m, run, 0)
    return int(run.max())


def _exact_numpy(mn, rd):
    """Exact fp32 reference scan (slow fallback; handles release events)."""
    Bn, Tn = mn.shape
    thr = np.float32(np.float32(rd) * np.float32(250.0))
    one = np.float32(1.0)
    note = np.zeros(Bn, np.float32)
    steps = np.zeros(Bn, np.float32)
    out = np.empty((Bn, Tn), np.float32)
    for t in range(Tn):
        x = mn[:, t]
        a = np.minimum(np.maximum(x, np.float32(0.0)), one)
        r = np.minimum(np.maximum(steps - thr, np.float32(0.0)), one)
        note = a * x + (one - a) * note * (one - r)
        steps = (steps + one) * (one - a) * (one - r)
        out[:, t] = note
    return out


def run(inputs, trace=False):
    """Run the Bass kernel on 8 cores. Returns (out [B,T] f32, BassKernelResults)."""
    mn = np.ascontiguousarray(np.asarray(inputs["midi_note"], dtype=np.float32))
    assert mn.shape == (B, T), f"expected {(B, T)}, got {mn.shape}"
    nc = _get_nc()
    in_maps = [
        {"x": np.ascontiguousarray(mn[c * LPC:(c + 1) * LPC])}
        for c in range(N_CORES)
    ]
    last_err = None
    for attempt in range(3):
        try:
            res = run_bass_kernel_spmd(nc, in_maps, list(range(N_CORES)),
                                       trace=trace)
            break
        except Exception as e:  # transient device wedge: reset + retry
            last_err = e
            if "UNRECOVERABLE" not in str(e) and "UNAVAILABLE" not in str(e):
                raise
            try:
                import ctypes
                lib = ctypes.CDLL("/opt/axon/libaxon_pjrt.so")
                lib.axon_reset.restype = ctypes.c_int64
                lib.axon_reset()
            except Exception:
                pass
    else:
        raise last_err
    out = np.concatenate([r["y"].astype(np.float32) for r in res.results],
                         axis=0)
    return out, res


def kernel(midi_note, release_duration):
    mn = np.asarray(midi_note, dtype=np.float32)
    rd = float(np.asarray(release_duration, dtype=np.float32))
    thr = rd * 250.0
    # Guard: linear-scan fast path is exact iff steps never exceeds thr,
    # which is guaranteed when every (x<1)-run is <= thr steps long.
    if _max_run_length_lt1(mn) > thr:
        return _exact_numpy(mn, rd)
    out, _ = run({"midi_note": mn})
    return out


# revision 46
# speedup vs baseline: 1.3601x; 1.3601x over previous
"""Trainium2 Bass kernel for nn_F0ProcessorCell — fused custom-DVE scan.

Reference semantics (per lane b, scanned over t):
    a_t = clamp(x_t, 0, 1)                      # note_activity
    r_t = clamp(s_{t-1} - thr, 0, 1)            # release_end, thr = rd*250
    n_t = a_t*x_t + (1-a_t)*n_{t-1}*(1-r_t)
    s_t = (s_{t-1}+1)*(1-a_t)*(1-r_t)
    out[b,t] = n_t

Fast path (guarded on CPU, exact): when every run of consecutive x<1 is
<= thr steps, r_t == 0 identically and the cell is the first-order
linear scan
    n_t = u_t*n_{t-1} + c_t,  a=clamp01(x), u=1-a, c=a*x.

This kernel computes THE WHOLE CELL in ONE hand-authored custom DVE
(VectorE) instruction per tile.  The stock pipeline needed ScalarE
relu/act + DVE STT + DVE tensor_tensor_scan (3.2 cyc/col); the fused op
reads raw f32 x and runs at the scan's intrinsic feedback rate
(2 cyc/col), writing fp16 directly:

uop program (v3, 8 ALU blocks; config travels WITH each element):
  uops[0] SEED   (1 cycle, no consume):  s4: flop4 <- CONST_0 (chunk carry)
  uops[1] ELEM A (consumes one x column):
      s0: r = MAX(x, 0)         s1: a = MIN(r, 1)
      s2: c = MULT(a, x)  [capture a -> lane4]
      s3: u = SUB(1, a)         s4: m = MULT(u, CURR=n_prev)
  uops[2] BUBBLE B (1 cycle, no consume, writes out):
      s2: BYPASS(CURR)=hold c   s3: BYPASS(PREV)=carry c
      s4: n = ADD(CURR=m, PREV=c)   s5..7: BYPASS -> WR0_LO (fp16)
A and B alternate: element e's MULT (cycle 2e+4) reads flop4=n_{e-1}
written by the previous bubble; bubble e's ADD (cycle 2e+5) combines
element e's m with the held c_e and rewrites flop4=n_e.  2 cycles per
element, exactly the stock scan's feedback rate, with all elementwise
prep riding the same pipeline for free.

y is stored fp16 (halves output HBM traffic; rel err ~1e-4 << the 2e-2
gate), upcast to f32 on host.  Falls back to an exact numpy scan if the
run-length guard fails.

Sharding: batch axis 0 (2048 lanes) split across 8 cores, 256 lanes per
core as 2 partition-groups of 128; time axis chunked, scan carry chained
across chunks via the previous out-tile's last column (fp16 [P,1] AP fed
to the seed uop through the CONST_0 slot).
"""

from dataclasses import dataclass

import numpy as np

from concourse import bacc, tile
from concourse import mybir
from concourse import dve_ops as _dve_ops_mod
from concourse.bass_utils import run_bass_kernel_spmd
from concourse.dve_ops import OPS, DveOp
from concourse.dve_spec import Spec, Src0, relu as _spec_relu
from concourse.dve_uop import (
    ENABLE,
    AluInp,
    AluOp,
    DelayInp,
    DveOpSpec,
    InpSel,
    OutPath,
    OutSel,
    Trigger,
    UopConfig,
)

N_CORES = 8
B, T = 2048, 16000
LPC = B // N_CORES          # 256 lanes per core
P = 128                     # SBUF partitions
GROUPS = LPC // P           # 2 partition-groups per core
F = 4000                    # max time-chunk (free-dim) size

_F32 = mybir.dt.float32
_F16 = mybir.dt.float16

_F0_NAME = "F0_FUSED_SCAN_ANT"

# delay lanes: 0=x(SRC_0) 1=ONE 2=ZERO 3=CONST_0(init) 4=a-capture
_LANES = (0, 1, 2, 3, 4, 5)
_PREV = AluInp.PREV_ALU_OUT
_CURR = AluInp.CURR_ALU_OUT


def _lane(d):
    return AluInp(int(AluInp.PREV_DELAY_0) + d)


def _mk_uop(inps, stages, capture=None, write=False, trigger=(), nxt=(0, 0, 0),
            repeat=0, consume=False, out_a_from=None):
    u = UopConfig()
    for d, sel in inps:
        u.enable_input(sel, d + 1)          # input lane d+1 feeds delay chain d
    for st in range(8):
        u.datapath_config[st].pass_through_delay(*_LANES)
    for st in range(8):
        op, a, b = stages.get(st, (AluOp.BYPASS, _PREV, _PREV))
        u.datapath_config[st].enable_alu(op, a, b)
    if capture is not None:
        st, ln = capture
        u.datapath_config[st].enable_delay_from_src(DelayInp.PREV_ALU_OUT, ln)
    if write:
        u.enable_output(OutSel.ALU_OUT, OutPath.WR0_LO)
    if out_a_from is not None:
        for st in range(out_a_from, 8):
            u.datapath_config[st].alu_out_a_enable = ENABLE
    u.accum_enabled = ENABLE   # accum_out carries the final scan state (f32)
    u.trigger = tuple(trigger)
    u.next_uop = tuple(nxt)
    u.repeat_count = repeat
    u.require_inp0 = int(consume)
    return u


def _build_f0_uops():
    # All three uops share one input-lane config (framework convention:
    # seed and steady have identical inp sets); only require_inp0 differs.
    inps = [(0, InpSel.SRC_0), (1, InpSel.ONE_F32), (2, InpSel.ZERO),
            (3, InpSel.CONST_0)]
    seed = _mk_uop(
        inps=inps,
        stages={4: (AluOp.BYPASS, _lane(3), _lane(3))},  # flop4 <- init
        trigger=(Trigger.COUNT, Trigger.NONE, Trigger.NONE),
        nxt=(1, 0, 0), repeat=1,
    )
    # Consuming slot: elementwise prep + u*state; ALSO emits the PREVIOUS
    # element's n (held in flop5 by the interleaved bubble slots) -> the
    # output stream is shifted one element; the caller pads in0 with one
    # dummy tail column and drops out column 0.
    elem = _mk_uop(
        inps=inps,
        stages={
            0: (AluOp.MAX, _lane(0), _lane(2)),        # r = max(x, 0)
            1: (AluOp.MIN, _PREV, _lane(1)),           # a = min(r, 1)
            2: (AluOp.MULTIPLY, _PREV, _lane(0)),      # c = a * x
            3: (AluOp.SUBTRACT, _lane(1), _lane(4)),   # u = 1 - a
            4: (AluOp.MULTIPLY, _PREV, _CURR),         # m = u * n_prev
            5: (AluOp.BYPASS, _CURR, _CURR),           # pick up held n_prev
        },
        capture=(2, 4),                                # lane4 <- a (flop1)
        write=True,
        trigger=(Trigger.SRC_TENSOR_DONE, Trigger.COUNT, Trigger.NONE),
        nxt=(0, 2, 0), repeat=1, consume=True,
    )
    # Non-consuming bubble (seed-shaped): completes n = m + c one cycle
    # behind the element and parks it in flop5 for the next element slot.
    # out_a threads n down the a-flops so the LAST bubble leaves the final
    # state in block 7's a-flop -> accum_out ([P,1] f32 chunk carry).
    bubble = _mk_uop(
        inps=inps,
        stages={
            2: (AluOp.BYPASS, _CURR, _CURR),           # hold c
            4: (AluOp.ADD, _CURR, _PREV),              # n = m + c
        },
        out_a_from=4,
        trigger=(Trigger.COUNT, Trigger.NONE, Trigger.NONE),
        nxt=(1, 0, 0), repeat=1,
    )
    return [seed, elem, bubble]


def _build_f0i_uops():
    """Interleaved dual-chain scan at 1 elem/cycle.

    The stream interleaves two independent recurrences (even/odd elements
    = chain A/B).  Stage 4 reads the scan state via NEXT_ALU_OUT_A =
    stage 5's A-flop, whose registered value is 2 elements old — exactly
    the same-chain previous state.  No bubble uop: every cycle consumes
    one element and writes one output.
      steady: s0 r=MAX(x,0); s1 a=MIN(r,1); s2 c=MULT(a,x)[cap a->L4];
              s3 u=SUB(1,a)[cap c->L5]; s4 m=MULT(u, NEXT_A);
              s5 n=ADD(m, L5(c)) -> A-flop5; s6/s7 BYPASS -> WR0_LO fp16
      seed1/seed2 (1 cycle each, no consume): s5 <- CONST_0 / CONST_1
    """
    # carries arrive IN-BAND on the Src1 stream (in1 = [P,2] f32): the two
    # seed slots each consume one element — the same mechanism the stock
    # framework uses for the C3 spill (latch-init consuming Src1 once).
    # CONST-slot APs are avoided entirely (dual-AP scalar delivery is
    # broken on HW: only the last AP's value lands).
    inps_seed = [(0, InpSel.SRC_0), (1, InpSel.ONE_F32), (2, InpSel.ZERO),
                 (3, InpSel.SRC_1)]
    inps_st = [(0, InpSel.SRC_0), (1, InpSel.ONE_F32), (2, InpSel.ZERO)]

    def seed(nxt_idx):
        u = _mk_uop(
            inps=inps_seed,
            stages={5: (AluOp.BYPASS, _lane(3), _lane(3))},
            trigger=(Trigger.COUNT, Trigger.NONE, Trigger.NONE),
            nxt=(nxt_idx, 0, 0), repeat=1,
        )
        u.require_inp1 = 1
        u.datapath_config[5].alu_out_a_enable = ENABLE
        return u

    steady = _mk_uop(
        inps=inps_st,
        stages={
            0: (AluOp.MAX, _lane(0), _lane(2)),
            1: (AluOp.MIN, _PREV, _lane(1)),
            2: (AluOp.MULTIPLY, _PREV, _lane(0)),
            3: (AluOp.SUBTRACT, _lane(1), _lane(4)),
            4: (AluOp.MULTIPLY, _PREV, AluInp.NEXT_ALU_OUT_A),
            5: (AluOp.ADD, _PREV, _lane(5)),
        },
        capture=(2, 4),
        write=True,
        trigger=(Trigger.SRC_TENSOR_DONE, Trigger.NONE, Trigger.NONE),
        nxt=(0, 0, 0), repeat=0, consume=True,
    )
    steady.datapath_config[3].enable_delay_from_src(DelayInp.PREV_ALU_OUT, 5)
    steady.datapath_config[5].alu_out_a_enable = ENABLE
    return [seed(1), seed(2), steady]


_F0I_NAME = "F0_FUSED_SCAN_ILV_ANT"


def _f0_reference(in0, in1, s0, s1, imm2):
    x = np.asarray(in0, np.float32)
    a = np.clip(x, 0.0, 1.0)
    u = 1.0 - a
    c = a * x
    state = np.broadcast_to(np.asarray(s0, np.float32).reshape(-1, 1),
                            (x.shape[0], 1)).copy().reshape(-1)
    out = np.empty_like(x)
    for t in range(x.shape[-1]):
        state = u[..., t] * state + c[..., t]
        out[..., t] = state
    return out, state.reshape(-1, 1)


@dataclass(frozen=True)
class _HandDveOp(DveOp):
    """DveOp whose uop program is hand-authored (bypasses lower()/sha)."""

    def compile(self, ver):
        from concourse.dve_ops import _COMPILE_CACHE, get_dve_sub_opcode
        key = (self.name, ver)
        if (r := _COMPILE_CACHE.get(key)) is not None:
            return r
        assert ver == "v3", f"{self.name} is hand-built for TRN2 (v3) only"
        builder = (_build_f0i_uops if self.name == _F0I_NAME
                   else _build_f0_uops)
        result = DveOpSpec(
            name=self.name,
            opcode=get_dve_sub_opcode(self.name),
            uops=builder(),
            rd1_en=(self.name == _F0I_NAME),
        )
        for u in result.uops:
            u.validate(ver)
        _COMPILE_CACHE[key] = result
        return result


def _register_op(name):
    for o in OPS:
        if o.name == name:
            return o
    op = _HandDveOp(
        name,
        # body is a registration placeholder (never lowered — compile() is
        # overridden); `reference` carries the true semantics for CoreSim.
        # accum=ADD marks the op accum_out-bearing (the final scan state).
        Spec(body=_spec_relu(Src0), accum=AluOp.ADD,
             reference=_f0_reference),
        False,
        {},
    )
    OPS.append(op)
    _dve_ops_mod._SUB_OPCODE_FOR_NAME[name] = (
        _dve_ops_mod._CUSTOM_DVE_ROW_BASE + len(OPS) - 1
    )
    assert _dve_ops_mod._SUB_OPCODE_FOR_NAME[name] < 0x20
    _dve_ops_mod.CUSTOM_DVE_SPECS[name] = op.spec
    return op


_F0_OP = _register_op(_F0_NAME)
_F0I_OP = _register_op(_F0I_NAME)


def _build_nc():
    nc = bacc.Bacc("TRN2", target_bir_lowering=False, debug=False,
                   num_devices=N_CORES)
    # interleaved layout: row p carries lanes p and p+128 interleaved
    # along time (even cols = lane p, odd = lane p+128), built on host.
    x_ap = nc.dram_tensor("x", [P, 2 * T], _F32, kind="ExternalInput").ap()
    y_ap = nc.dram_tensor("y", [P, 2 * T], _F16, kind="ExternalOutput").ap()

    with tile.TileContext(nc) as tc:
        with (
            tc.tile_pool(name="xin", bufs=7) as pool_x,
            tc.tile_pool(name="nout", bufs=5) as pool_n,
            tc.tile_pool(name="carry", bufs=3) as pool_cr,
        ):
            from collections import deque
            prev = [None]
            pend = deque()

            # widths in interleaved columns (2x time steps)
            widths = [1000, 1000, 2000] + [4000] * 6 + [2000, 1000, 1000]
            assert sum(widths) == 2 * T
            segs, off = [], 0
            for w in widths:
                segs.append((off, w))
                off += w

            def emit_front(seg):
                off, w = seg
                xt = pool_x.tile([P, F], _F32, tag="x")
                nc.sync.dma_start(xt[:, 0:w], x_ap[:, off:off + w])
                pend.append((xt, seg))

            def emit_back():
                xt, (off, w) = pend.popleft()
                nt = pool_n.tile([P, F], _F16, tag="n")
                if prev[0] is None:
                    z = pool_cr.tile([P, 2], _F32, tag="carry")
                    nc.vector.memset(z[:, :], 0.0)
                    prev[0] = z
                nc.vector._custom_dve(_F0I_OP, out=nt[:, 0:w],
                                      in0=xt[:, 0:w], in1=prev[0][:, 0:2])
                # chunk carries = last two outputs (chain A, chain B);
                # upconvert to the f32 the ISA scalar slots require
                ct = pool_cr.tile([P, 2], _F32, tag="carry")
                nc.vector.tensor_copy(ct[:, :], nt[:, w - 2:w])
                prev[0] = ct
                nc.sync.dma_start(y_ap[:, off:off + w], nt[:, 0:w])

            LAG = 3
            NSEG = len(segs)
            for k in range(NSEG + LAG):
                if k >= LAG:
                    emit_back()
                if k < NSEG:
                    emit_front(segs[k])
    nc.compile()
    return nc


_NC_CACHE = None


def _get_nc():
    global _NC_CACHE
    if _NC_CACHE is None:
        _NC_CACHE = _build_nc()
    return _NC_CACHE


def _max_run_length_lt1(x):
    """Max length, over all lanes, of a run of consecutive values < 1.0."""
    m = x < np.float32(1.0)                      # [B, T] bool
    cs = np.cumsum(m, axis=1, dtype=np.int64)
    reset = np.where(~m, cs, 0)
    run = cs - np.maximum.accumulate(reset, axis=1)
    run = np.where(m, run, 0)
    return int(run.max())


def _exact_numpy(mn, rd):
    """Exact fp32 reference scan (slow fallback; handles release events)."""
    Bn, Tn = mn.shape
    thr = np.float32(np.float32(rd) * np.float32(250.0))
    one = np.float32(1.0)
    note = np.zeros(Bn, np.float32)
    steps = np.zeros(Bn, np.float32)
    out = np.empty((Bn, Tn), np.float32)
    for t in range(Tn):
        x = mn[:, t]
        a = np.minimum(np.maximum(x, np.float32(0.0)), one)
        r = np.minimum(np.maximum(steps - thr, np.float32(0.0)), one)
        note = a * x + (one - a) * note * (one - r)
        steps = (steps + one) * (one - a) * (one - r)
        out[:, t] = note
    return out


def run(inputs, trace=False):
    """Run the Bass kernel on 8 cores. Returns (out [B,T] f32, BassKernelResults)."""
    mn = np.ascontiguousarray(np.asarray(inputs["midi_note"], dtype=np.float32))
    assert mn.shape == (B, T), f"expected {(B, T)}, got {mn.shape}"
    nc = _get_nc()
    in_maps = []
    for c in range(N_CORES):
        blk = mn[c * LPC:(c + 1) * LPC]
        xi = np.empty((P, 2 * T), np.float32)
        xi[:, 0::2] = blk[0:P]
        xi[:, 1::2] = blk[P:LPC]
        in_maps.append({"x": xi})
    last_err = None
    for attempt in range(3):
        try:
            res = run_bass_kernel_spmd(nc, in_maps, list(range(N_CORES)),
                                       trace=trace)
            break
        except Exception as e:  # transient device wedge: reset + retry
            last_err = e
            if "UNRECOVERABLE" not in str(e) and "UNAVAILABLE" not in str(e):
                raise
            try:
                import ctypes
                lib = ctypes.CDLL("/opt/axon/libaxon_pjrt.so")
                lib.axon_reset.restype = ctypes.c_int64
                lib.axon_reset()
            except Exception:
                pass
    else:
        raise last_err
    out = np.empty((B, T), np.float32)
    for c, r in enumerate(res.results):
        yi = r["y"].astype(np.float32)
        out[c * LPC:c * LPC + P] = yi[:, 0::2]
        out[c * LPC + P:(c + 1) * LPC] = yi[:, 1::2]
    return out, res


def kernel(midi_note, release_duration):
    mn = np.asarray(midi_note, dtype=np.float32)
    rd = float(np.asarray(release_duration, dtype=np.float32))
    thr = rd * 250.0
    # Guard: linear-scan fast path is exact iff steps never exceeds thr,
    # which is guaranteed when every (x<1)-run is <= thr steps long.
    if _max_run_length_lt1(mn) > thr:
        return _exact_numpy(mn, rd)
    out, _ = run({"midi_note": mn})
    return out
